# revision 3
# baseline (speedup 1.0000x reference)
"""Trainium2 Bass kernel for nn_ConvThreshold — V16: separable Horner, 2-phase, no gpsimd.

conv[p] = sum_{dy,dx in [-2,2]} relu(x)[p+(dy,dx)] * t[p]^(dy^2+dx^2),
t[p] = exp(-1/(2*scale[p]^2)); mask = conv >= 0.5.

Key identity: t^(dy^2+dx^2) = t^(dy^2) * t^(dx^2) (separable), so with
vertical ring sums u1 = x(r-1)+x(r+1), u2 = x(r-2)+x(r+2) (free-dim
shifted adds on DVE) the 5 ring groups need only 8 banded matmuls:
  P1 = b1*u0 + u1   (t^1)     P2 = b1*u1        (t^2)
  P4 = b2*u0 + u2   (t^4)     P5 = b2*u1 + b1*u2 (t^5)
  P8 = b2*u2        (t^8)
(11 matmuls/stripe vs 16 in the non-separable form.)
Horner: conv = u0 + t*(P1 + t*P2) + t^4*(P4 + t*P5 + t^4*P8), done via
3 fold matmuls (id*m1 -> P1, id*m2, id*m3 -> P4) where [m1|m2] = t1*[e2|e5]
(one stride-0-broadcast DVE mul), m3 = t4*e8. [P2|P5|P8] evacuated by one
packed ScalarE copy; [P1'|P4'] read straight from PSUM by the fused
mab = [t1|t4]*[A|B] DVE mul; s = A'+B'; oc = u0+s; mask = (oc>=0.5) in fp8.
u1,u2 are ONE packed DVE add via a stride-trick AP (segments at offsets
1/0 and 3/4). All elementwise work stays on DVE+Act (gpsimd measured
~9us slower for 2 ops/stripe on HW; 3-deep phase pipelining and
whole-image-wide packed ops also measured slower - they serialize the
per-stripe chain). 2-phase pipeline: phase1(k+1) overlaps phase2(k).

Layout: partition = image COLUMN, free = ROW; 8 cores x (image, H-half);
7 overlapping 128-column stripes (124 valid cols each).
"""

import sys

sys.path.insert(0, "/opt/trn_rl_repo")

from contextlib import ExitStack

import numpy as np

import concourse.bass as bass
import concourse.tile as tile
from concourse import bacc, mybir
from concourse.bass_utils import run_bass_kernel_spmd

F32 = mybir.dt.float32
F16 = mybir.dt.float16
F8 = mybir.dt.float8e4

B, H, W = 4, 768, 768
NCORES = 8
SLAB = H // 2
NST = 7
CW = 124
XR = SLAB + 4          # 388
SEG = XR + SLAB        # 772
ROWS = SLAB            # 384
BANK = 512

_CACHE = {}


def _consts():
    ident = np.eye(128, dtype=np.float16)
    b1 = np.zeros((128, 128), dtype=np.float16)
    b2 = np.zeros((128, 128), dtype=np.float16)
    for m in range(128):
        for d in (-1, 1):
            if 0 <= m + d < 128:
                b1[m + d, m] = 1.0
        for d in (-2, 2):
            if 0 <= m + d < 128:
                b2[m + d, m] = 1.0
    wpack = np.concatenate([b1, b2, ident], axis=1)
    return {"wpack": wpack}


def _seg2(view, off0, step, n=ROWS):
    """AP over `view`'s tensor: two segments of length n at elem offsets
    off0 and off0+step (outer dim count 2, stride `step`)."""
    base = view[:, off0 : off0 + n]
    return bass.AP(
        base.tensor, base.offset, [list(base.ap[0]), [step, 2], [1, n]]
    )


def _bcast2(view, n=ROWS):
    """[128, n] view broadcast to [128, 2, n] via stride-0 outer dim."""
    return bass.AP(view.tensor, view.offset, [list(view.ap[0]), [0, 2], [1, n]])


def _build(repeat: int = 1):
    nc = bacc.Bacc(
        "TRN2",
        target_bir_lowering=False,
        debug=False,
        enable_asserts=True,
        num_devices=NCORES,
    )
    ins_d = nc.dram_tensor("ins", [128, NST * SEG], F16, kind="ExternalInput").ap()
    wp_d = nc.dram_tensor("wpack", [128, 384], F16, kind="ExternalInput").ap()
    conv_d = nc.dram_tensor("conv", [128, NST * ROWS], F16, kind="ExternalOutput").ap()
    mask_d = nc.dram_tensor("mask", [128, NST * ROWS], F8, kind="ExternalOutput").ap()

    with tile.TileContext(nc, trace_sim=False) as tc, ExitStack() as ctx:
        sb = ctx.enter_context(tc.tile_pool(name="sb", bufs=3))
        cb = ctx.enter_context(tc.tile_pool(name="cb", bufs=1))
        ps = ctx.enter_context(tc.tile_pool(name="ps", bufs=1, space="PSUM"))

        wp = cb.tile([128, 384], F16, tag="wpack")
        nc.sync.dma_start(wp[:], wp_d[:])
        w_b1, w_b2, w_id = wp[:, 0:128], wp[:, 128:256], wp[:, 256:384]

        def _body():
            ins = sb.tile([128, NST * SEG], F16, tag="ins")
            for k in range(0, NST, 2):
                o, o2 = k * SEG, min(NST, k + 2) * SEG
                nc.sync.dma_start(ins[:, o:o2], ins_d[:, o:o2])
            oc = sb.tile([128, NST * ROWS], F16, tag="oc")
            om = sb.tile([128, NST * ROWS], F8, tag="om")

            st = [dict() for _ in range(NST)]

            def phase1(k):
                o = k * SEG
                xk = ins[:, o : o + XR]
                sk = ins[:, o + XR : o + SEG]
                d = st[k]

                xr = sb.tile([128, XR], F16, tag=f"xr{k%2}")
                nc.vector.tensor_scalar_max(xr[:], xk, 0.0)
                d["xr"] = xr
                u0c = xr[:, 2 : 2 + ROWS]

                # u12 = [u1|u2] in one packed DVE add:
                #   u1 = xr[1:385] + xr[3:387]; u2 = xr[0:384] + xr[4:388]
                u12 = sb.tile([128, 2 * ROWS], F16, tag=f"u12{k%2}")
                nc.vector.tensor_add(
                    u12[:].rearrange("p (b r) -> p b r", b=2, r=ROWS),
                    _seg2(xr, 1, -1),
                    _seg2(xr, 3, 1),
                )
                u1, u2 = u12[:, 0:ROWS], u12[:, ROWS : 2 * ROWS]

                # weights: vr = 1/s^2; t1 = exp(-0.5 vr); t4 = exp(-2 vr)
                q = sb.tile([128, ROWS], F32, tag=f"q{k%2}")
                nc.scalar.activation(q[:], sk, mybir.ActivationFunctionType.Square)
                vr = sb.tile([128, ROWS], F32, tag=f"vr{k%2}")
                nc.vector.reciprocal_approx_fast(vr[:], q[:])
                t14 = sb.tile([128, 2 * ROWS], F16, tag=f"t14{k%2}")
                d["t14"] = t14
                nc.scalar.activation(
                    t14[:, 0:ROWS], vr[:], mybir.ActivationFunctionType.Exp,
                    scale=-0.5,
                )
                nc.scalar.activation(
                    t14[:, ROWS : 2 * ROWS], vr[:],
                    mybir.ActivationFunctionType.Exp, scale=-2.0,
                )

                pair = ps.tile([128, 2 * BANK], F32, tag=f"pair{k%2}")
                trip = ps.tile([128, 3 * BANK], F32, tag="trip")
                p1 = pair[:, 0:ROWS]
                p4 = pair[:, BANK : BANK + ROWS]
                p2 = trip[:, 0:ROWS]
                p5 = trip[:, BANK : BANK + ROWS]
                p8 = trip[:, 2 * BANK : 2 * BANK + ROWS]
                d.update(pair=pair, trip=trip, p1=p1, p4=p4)

                nc.tensor.matmul(p1, w_b1, u0c, start=True, stop=False)
                nc.tensor.matmul(p1, w_id, u1, start=False, stop=False)
                nc.tensor.matmul(p2, w_b1, u1, start=True, stop=True)
                nc.tensor.matmul(p4, w_b2, u0c, start=True, stop=False)
                nc.tensor.matmul(p4, w_id, u2, start=False, stop=False)
                nc.tensor.matmul(p5, w_b2, u1, start=True, stop=False)
                nc.tensor.matmul(p5, w_b1, u2, start=False, stop=True)
                nc.tensor.matmul(p8, w_b2, u2, start=True, stop=True)

            def phase2a(k):
                d = st[k]
                t14 = d["t14"]
                t1 = t14[:, 0:ROWS]
                t4 = t14[:, ROWS : 2 * ROWS]

                # packed ScalarE evac of [P2|P5|P8]
                e = sb.tile([128, 3 * ROWS], F16, tag=f"e{k%2}")
                nc.scalar.copy(
                    e[:].rearrange("p (b r) -> p b r", b=3, r=ROWS),
                    d["trip"][:].rearrange("p (b r) -> p b r", b=3, r=BANK)[
                        :, :, 0:ROWS
                    ],
                )

                # m12 = t1 (x) [e2|e5] in one bcast DVE mul
                m12 = sb.tile([128, 2 * ROWS], F16, tag=f"m12{k%2}")
                nc.vector.tensor_mul(
                    m12[:].rearrange("p (b r) -> p b r", b=2, r=ROWS),
                    _bcast2(t1),
                    e[:, 0 : 2 * ROWS].rearrange("p (b r) -> p b r", b=2, r=ROWS),
                )
                # m3 = t4 * e8 on gpsimd
                m3 = sb.tile([128, ROWS], F16, tag=f"m3{k%2}")
                nc.vector.tensor_mul(m3[:], t4, e[:, 2 * ROWS : 3 * ROWS])

                # folds: P1 += m1; P4 += m2 + m3
                nc.tensor.matmul(d["p1"], w_id, m12[:, 0:ROWS], start=False, stop=True)
                nc.tensor.matmul(d["p4"], w_id, m12[:, ROWS : 2 * ROWS],
                                 start=False, stop=False)
                nc.tensor.matmul(d["p4"], w_id, m3[:], start=False, stop=True)

            def phase2b(k):
                d = st[k]
                t14 = d["t14"]
                xr = d["xr"]
                u0c = xr[:, 2 : 2 + ROWS]

                # fused evac+mul: mab = [t1|t4] (x) [P1'|P4'] straight from PSUM
                mab = sb.tile([128, 2 * ROWS], F16, tag=f"mab{k%2}")
                nc.vector.tensor_mul(
                    mab[:].rearrange("p (b r) -> p b r", b=2, r=ROWS),
                    t14[:].rearrange("p (b r) -> p b r", b=2, r=ROWS),
                    d["pair"][:].rearrange("p (b r) -> p b r", b=2, r=BANK)[
                        :, :, 0:ROWS
                    ],
                )
                s = sb.tile([128, ROWS], F16, tag=f"s{k%2}")
                nc.vector.tensor_add(s[:], mab[:, 0:ROWS], mab[:, ROWS : 2 * ROWS])

                co = k * ROWS
                nc.vector.tensor_add(oc[:, co : co + ROWS], u0c, s[:])
                nc.vector.tensor_scalar(
                    om[:, co : co + ROWS], oc[:, co : co + ROWS],
                    0.5, None, mybir.AluOpType.is_ge,
                )

            def flush(lo, hi):
                a, b = lo * ROWS, hi * ROWS
                nc.scalar.dma_start(conv_d[:, a:b], oc[:, a:b])
                nc.scalar.dma_start(mask_d[:, a:b], om[:, a:b])

            phase1(0)
            for k in range(NST):
                if k + 1 < NST:
                    phase1(k + 1)
                phase2a(k)
                phase2b(k)

            for k in range(0, NST, 2):
                flush(k, min(k + 2, NST))

        if repeat == 1:
            _body()
        elif repeat % 2 == 0:
            with tc.For_i(0, repeat // 2, 1):
                _body()
                _body()
        else:
            with tc.For_i(0, repeat, 1):
                _body()

    nc.compile()
    return nc


def make_in_maps(bev_map: np.ndarray, bev_scale: np.ndarray):
    consts = _consts()
    in_maps = []
    for c in range(NCORES):
        b, hh = c // 2, c % 2
        xT = np.zeros((124 * NST + 4, 772), dtype=np.float16)
        xT[2:770, 2:770] = bev_map[b, 0].T
        sT = np.ones((124 * NST + 4, 768), dtype=np.float16)
        sT[2:770, :] = bev_scale[b, 0].T
        r0 = hh * SLAB
        ins = np.empty((128, NST * SEG), dtype=np.float16)
        for k in range(NST):
            o = k * SEG
            ins[:, o : o + XR] = xT[124 * k : 124 * k + 128, r0 : r0 + XR]
            ins[:, o + XR : o + SEG] = sT[
                124 * k : 124 * k + 128, r0 : r0 + SLAB
            ]
        m = {"ins": ins}
        m.update({k2: v.copy() for k2, v in consts.items()})
        in_maps.append(m)
    return in_maps


def _unpack(res):
    conv = np.empty((B, 1, H, W), dtype=np.float32)
    mask = np.empty((B, 1, H, W), dtype=np.float32)
    for c in range(NCORES):
        b, hh = c // 2, c % 2
        ocf = np.asarray(res[c]["conv"]).astype(np.float32)
        omf = np.asarray(res[c]["mask"]).astype(np.float32)
        convT = np.empty((W, SLAB), dtype=np.float32)
        maskT = np.empty((W, SLAB), dtype=np.float32)
        for k in range(NST):
            c0 = 124 * k
            n = min(CW, W - c0)
            seg = slice(k * ROWS, (k + 1) * ROWS)
            convT[c0 : c0 + n] = ocf[2 : 2 + n, seg]
            maskT[c0 : c0 + n] = omf[2 : 2 + n, seg]
        r0 = hh * SLAB
        conv[b, 0, r0 : r0 + SLAB, :] = convT.T
        mask[b, 0, r0 : r0 + SLAB, :] = maskT.T
    return conv, mask


def kernel(bev_map: np.ndarray, bev_scale: np.ndarray):
    assert bev_map.shape == (B, 1, H, W) and bev_scale.shape == (B, 1, H, W)
    if "nc" not in _CACHE:
        _CACHE["nc"] = _build()
    nc = _CACHE["nc"]
    in_maps = make_in_maps(bev_map, bev_scale)
    res = run_bass_kernel_spmd(nc, in_maps, list(range(NCORES))).results
    return _unpack(res)


# revision 6
# speedup vs baseline: 1.0537x; 1.0537x over previous
"""Trainium2 Bass kernel for nn_ConvThreshold — V28: separable Horner.

conv[p] = sum_{dy,dx in [-2,2]} relu(x)[p+(dy,dx)] * t[p]^(dy^2+dx^2),
t[p] = exp(-1/(2*scale[p]^2)); mask = conv >= 0.5 (f16).

Key identity: t^(dy^2+dx^2) = t^(dy^2) * t^(dx^2) (separable), so with
vertical ring sums u1 = x(r-1)+x(r+1), u2 = x(r-2)+x(r+2) (free-dim
shifted adds on DVE) the 5 ring groups need only 8 banded matmuls
(vs 16 non-separable):
  P1 = b1*u0 + u1   (t^1)     P2 = b1*u1        (t^2)
  P4 = b2*u0 + u2   (t^4)     P5 = b2*u1 + b1*u2 (t^5)
  P8 = b2*u2        (t^8)
Horner: conv = u0 + t*(P1 + t*P2) + t^4*(P4 + t*P5 + t^4*P8), via 3 fold
matmuls (id*m1 -> P1; id*m2, id*m3 -> P4) with m1 = t1*e2, m2 = t1*e5,
m3 = t4*e8. [P2|P5|P8] evacuated by one packed ScalarE copy; final
mabA = t1*P1', mabB = t4*P4' read PSUM directly; oc = u0 + (mabA+mabB);
mask = oc >= 0.5 on DVE (f16 out = 4x mode). 2-phase software pipeline:
phase1(k+1) overlaps phase2(k).

HW-measured mode lessons baked in: every DVE op uses plain 2-operand
views with 4B-aligned even bases where possible — packed multi-segment
APs (stride-0 broadcast, odd-offset segment pairs, multi-PSUM-bank
views) and f8 outputs all fall off the 2x/4x fast paths on real HW, and
gpsimd tensor ops are far slower than the cost model claims.

Layout: partition = image COLUMN, free = ROW; 8 cores x (image, H-half);
7 overlapping 128-column stripes (124 valid cols each).
"""

import sys

sys.path.insert(0, "/opt/trn_rl_repo")

from contextlib import ExitStack

import numpy as np

import concourse.bass as bass
import concourse.tile as tile
from concourse import bacc, mybir
from concourse.bass_utils import run_bass_kernel_spmd

F32 = mybir.dt.float32
F16 = mybir.dt.float16
F8 = mybir.dt.float8e4

B, H, W = 4, 768, 768
NCORES = 8
SLAB = H // 2
NST = 7
CW = 124
XR = SLAB + 4          # 388
SEG = XR + SLAB        # 772
ROWS = SLAB            # 384
BANK = 512

_CACHE = {}


def _consts():
    ident = np.eye(128, dtype=np.float16)
    b1 = np.zeros((128, 128), dtype=np.float16)
    b2 = np.zeros((128, 128), dtype=np.float16)
    for m in range(128):
        for d in (-1, 1):
            if 0 <= m + d < 128:
                b1[m + d, m] = 1.0
        for d in (-2, 2):
            if 0 <= m + d < 128:
                b2[m + d, m] = 1.0
    wpack = np.concatenate([b1, b2, ident], axis=1)
    return {"wpack": wpack}


def _seg2(view, off0, step, n=ROWS):
    """AP over `view`'s tensor: two segments of length n at elem offsets
    off0 and off0+step (outer dim count 2, stride `step`)."""
    base = view[:, off0 : off0 + n]
    return bass.AP(
        base.tensor, base.offset, [list(base.ap[0]), [step, 2], [1, n]]
    )


def _bcast2(view, n=ROWS):
    """[128, n] view broadcast to [128, 2, n] via stride-0 outer dim."""
    return bass.AP(view.tensor, view.offset, [list(view.ap[0]), [0, 2], [1, n]])


def _build(repeat: int = 1):
    nc = bacc.Bacc(
        "TRN2",
        target_bir_lowering=False,
        debug=False,
        enable_asserts=True,
        num_devices=NCORES,
    )
    ins_d = nc.dram_tensor("ins", [128, NST * SEG], F16, kind="ExternalInput").ap()
    wp_d = nc.dram_tensor("wpack", [128, 384], F16, kind="ExternalInput").ap()
    conv_d = nc.dram_tensor("conv", [128, NST * ROWS], F16, kind="ExternalOutput").ap()
    mask_d = nc.dram_tensor("mask", [128, NST * ROWS], F16, kind="ExternalOutput").ap()

    with tile.TileContext(nc, trace_sim=False) as tc, ExitStack() as ctx:
        sb = ctx.enter_context(tc.tile_pool(name="sb", bufs=3))
        cb = ctx.enter_context(tc.tile_pool(name="cb", bufs=1))
        ps = ctx.enter_context(tc.tile_pool(name="ps", bufs=1, space="PSUM"))

        wp = cb.tile([128, 384], F16, tag="wpack")
        nc.sync.dma_start(wp[:], wp_d[:])
        w_b1, w_b2, w_id = wp[:, 0:128], wp[:, 128:256], wp[:, 256:384]

        def _body():
            ins = sb.tile([128, NST * SEG], F16, tag="ins")
            for k in range(0, NST, 2):
                o, o2 = k * SEG, min(NST, k + 2) * SEG
                nc.sync.dma_start(ins[:, o:o2], ins_d[:, o:o2])
            oc = sb.tile([128, NST * ROWS], F16, tag="oc")
            om = sb.tile([128, NST * ROWS], F16, tag="om")

            st = [dict() for _ in range(NST)]

            def phase1(k):
                o = k * SEG
                xk = ins[:, o : o + XR]
                sk = ins[:, o + XR : o + SEG]
                d = st[k]

                xr = sb.tile([128, XR], F16, tag=f"xr{k%2}")
                nc.vector.tensor_scalar_max(xr[:], xk, 0.0)
                d["xr"] = xr
                u0c = xr[:, 2 : 2 + ROWS]

                # u12 = [u1|u2] in one packed DVE add:
                #   u1 = xr[1:385] + xr[3:387]; u2 = xr[0:384] + xr[4:388]
                u12 = sb.tile([128, 2 * ROWS], F16, tag=f"u12{k%2}")
                nc.vector.tensor_add(u12[:, 0:ROWS], xr[:, 1 : 1 + ROWS],
                                     xr[:, 3 : 3 + ROWS])
                nc.vector.tensor_add(u12[:, ROWS : 2 * ROWS], xr[:, 0:ROWS],
                                     xr[:, 4 : 4 + ROWS])
                u1, u2 = u12[:, 0:ROWS], u12[:, ROWS : 2 * ROWS]

                # weights: vr = 1/s^2; t1 = exp(-0.5 vr); t4 = exp(-2 vr)
                q = sb.tile([128, ROWS], F32, tag=f"q{k%2}")
                nc.scalar.activation(q[:], sk, mybir.ActivationFunctionType.Square)
                vr = sb.tile([128, ROWS], F32, tag=f"vr{k%2}")
                nc.vector.reciprocal_approx_fast(vr[:], q[:])
                t14 = sb.tile([128, 2 * ROWS], F16, tag=f"t14{k%2}")
                d["t14"] = t14
                nc.scalar.activation(
                    t14[:, 0:ROWS], vr[:], mybir.ActivationFunctionType.Exp,
                    scale=-0.5,
                )
                nc.scalar.activation(
                    t14[:, ROWS : 2 * ROWS], vr[:],
                    mybir.ActivationFunctionType.Exp, scale=-2.0,
                )

                pair = ps.tile([128, 2 * BANK], F32, tag=f"pair{k%2}")
                trip = ps.tile([128, 3 * BANK], F32, tag="trip")
                p1 = pair[:, 0:ROWS]
                p4 = pair[:, BANK : BANK + ROWS]
                p2 = trip[:, 0:ROWS]
                p5 = trip[:, BANK : BANK + ROWS]
                p8 = trip[:, 2 * BANK : 2 * BANK + ROWS]
                d.update(pair=pair, trip=trip, p1=p1, p4=p4)

                nc.tensor.matmul(p1, w_b1, u0c, start=True, stop=False)
                nc.tensor.matmul(p1, w_id, u1, start=False, stop=False)
                nc.tensor.matmul(p2, w_b1, u1, start=True, stop=True)
                nc.tensor.matmul(p4, w_b2, u0c, start=True, stop=False)
                nc.tensor.matmul(p4, w_id, u2, start=False, stop=False)
                nc.tensor.matmul(p5, w_b2, u1, start=True, stop=False)
                nc.tensor.matmul(p5, w_b1, u2, start=False, stop=True)
                nc.tensor.matmul(p8, w_b2, u2, start=True, stop=True)

            def phase2a(k):
                d = st[k]
                t14 = d["t14"]
                t1 = t14[:, 0:ROWS]
                t4 = t14[:, ROWS : 2 * ROWS]

                # packed ScalarE evac of [P2|P5|P8]
                e = sb.tile([128, 3 * ROWS], F16, tag=f"e{k%2}")
                nc.scalar.copy(
                    e[:].rearrange("p (b r) -> p b r", b=3, r=ROWS),
                    d["trip"][:].rearrange("p (b r) -> p b r", b=3, r=BANK)[
                        :, :, 0:ROWS
                    ],
                )

                # m1 = t1*e2; m2 = t1*e5 (plain 2x muls)
                m12 = sb.tile([128, 2 * ROWS], F16, tag=f"m12{k%2}")
                nc.vector.tensor_mul(m12[:, 0:ROWS], t1, e[:, 0:ROWS])
                nc.vector.tensor_mul(m12[:, ROWS : 2 * ROWS], t1,
                                     e[:, ROWS : 2 * ROWS])
                # m3 = t4 * e8 on gpsimd
                m3 = sb.tile([128, ROWS], F16, tag=f"m3{k%2}")
                nc.vector.tensor_mul(m3[:], t4, e[:, 2 * ROWS : 3 * ROWS])

                # folds: P1 += m1; P4 += m2 + m3
                nc.tensor.matmul(d["p1"], w_id, m12[:, 0:ROWS], start=False, stop=True)
                nc.tensor.matmul(d["p4"], w_id, m12[:, ROWS : 2 * ROWS],
                                 start=False, stop=False)
                nc.tensor.matmul(d["p4"], w_id, m3[:], start=False, stop=True)

            def phase2b(k):
                d = st[k]
                t14 = d["t14"]
                xr = d["xr"]
                u0c = xr[:, 2 : 2 + ROWS]

                # fused evac+mul: mab = [t1|t4] (x) [P1'|P4'] straight from PSUM
                mab = sb.tile([128, 2 * ROWS], F16, tag=f"mab{k%2}")
                nc.vector.tensor_mul(
                    mab[:].rearrange("p (b r) -> p b r", b=2, r=ROWS),
                    t14[:].rearrange("p (b r) -> p b r", b=2, r=ROWS),
                    d["pair"][:].rearrange("p (b r) -> p b r", b=2, r=BANK)[
                        :, :, 0:ROWS
                    ],
                )
                s = sb.tile([128, ROWS], F16, tag=f"s{k%2}")
                nc.vector.tensor_add(s[:], mab[:, 0:ROWS], mab[:, ROWS : 2 * ROWS])

                co = k * ROWS
                nc.vector.tensor_add(oc[:, co : co + ROWS], u0c, s[:])
                nc.vector.tensor_scalar(
                    om[:, co : co + ROWS], oc[:, co : co + ROWS],
                    0.5, None, mybir.AluOpType.is_ge,
                )

            def flush(lo, hi):
                a, b = lo * ROWS, hi * ROWS
                nc.scalar.dma_start(conv_d[:, a:b], oc[:, a:b])
                nc.scalar.dma_start(mask_d[:, a:b], om[:, a:b])

            phase1(0)
            for k in range(NST):
                if k + 1 < NST:
                    phase1(k + 1)
                phase2a(k)
                phase2b(k)

            for k in range(0, NST, 2):
                flush(k, min(k + 2, NST))

        if repeat == 1:
            _body()
        elif repeat % 2 == 0:
            with tc.For_i(0, repeat // 2, 1):
                _body()
                _body()
        else:
            with tc.For_i(0, repeat, 1):
                _body()

    nc.compile()
    return nc


def make_in_maps(bev_map: np.ndarray, bev_scale: np.ndarray):
    consts = _consts()
    in_maps = []
    for c in range(NCORES):
        b, hh = c // 2, c % 2
        xT = np.zeros((124 * NST + 4, 772), dtype=np.float16)
        xT[2:770, 2:770] = bev_map[b, 0].T
        sT = np.ones((124 * NST + 4, 768), dtype=np.float16)
        sT[2:770, :] = bev_scale[b, 0].T
        r0 = hh * SLAB
        ins = np.empty((128, NST * SEG), dtype=np.float16)
        for k in range(NST):
            o = k * SEG
            ins[:, o : o + XR] = xT[124 * k : 124 * k + 128, r0 : r0 + XR]
            ins[:, o + XR : o + SEG] = sT[
                124 * k : 124 * k + 128, r0 : r0 + SLAB
            ]
        m = {"ins": ins}
        m.update({k2: v.copy() for k2, v in consts.items()})
        in_maps.append(m)
    return in_maps


def _unpack(res):
    conv = np.empty((B, 1, H, W), dtype=np.float32)
    mask = np.empty((B, 1, H, W), dtype=np.float32)
    for c in range(NCORES):
        b, hh = c // 2, c % 2
        ocf = np.asarray(res[c]["conv"]).astype(np.float32)
        omf = np.asarray(res[c]["mask"]).astype(np.float32)
        convT = np.empty((W, SLAB), dtype=np.float32)
        maskT = np.empty((W, SLAB), dtype=np.float32)
        for k in range(NST):
            c0 = 124 * k
            n = min(CW, W - c0)
            seg = slice(k * ROWS, (k + 1) * ROWS)
            convT[c0 : c0 + n] = ocf[2 : 2 + n, seg]
            maskT[c0 : c0 + n] = omf[2 : 2 + n, seg]
        r0 = hh * SLAB
        conv[b, 0, r0 : r0 + SLAB, :] = convT.T
        mask[b, 0, r0 : r0 + SLAB, :] = maskT.T
    return conv, mask


def kernel(bev_map: np.ndarray, bev_scale: np.ndarray):
    assert bev_map.shape == (B, 1, H, W) and bev_scale.shape == (B, 1, H, W)
    if "nc" not in _CACHE:
        _CACHE["nc"] = _build()
    nc = _CACHE["nc"]
    in_maps = make_in_maps(bev_map, bev_scale)
    res = run_bass_kernel_spmd(nc, in_maps, list(range(NCORES))).results
    return _unpack(res)


# revision 7
# speedup vs baseline: 1.1532x; 1.0944x over previous
"""Trainium2 Bass kernel for nn_ConvThreshold — V28: separable Horner.

conv[p] = sum_{dy,dx in [-2,2]} relu(x)[p+(dy,dx)] * t[p]^(dy^2+dx^2),
t[p] = exp(-1/(2*scale[p]^2)); mask = conv >= 0.5 (f16).

Key identity: t^(dy^2+dx^2) = t^(dy^2) * t^(dx^2) (separable), so with
vertical ring sums u1 = x(r-1)+x(r+1), u2 = x(r-2)+x(r+2) (free-dim
shifted adds on DVE) the 5 ring groups need only 8 banded matmuls
(vs 16 non-separable):
  P1 = b1*u0 + u1   (t^1)     P2 = b1*u1        (t^2)
  P4 = b2*u0 + u2   (t^4)     P5 = b2*u1 + b1*u2 (t^5)
  P8 = b2*u2        (t^8)
Horner: conv = u0 + t*(P1 + t*P2) + t^4*(P4 + t*P5 + t^4*P8), via 3 fold
matmuls (id*m1 -> P1; id*m2, id*m3 -> P4) with m1 = t1*e2, m2 = t1*e5,
m3 = t4*e8. [P2|P5|P8] evacuated by one packed ScalarE copy; final
mab = [t1|t4]*[P1'|P4'] reads PSUM directly (packed); oc = u0 + (mabA+mabB);
mask = oc >= 0.5 on DVE (f16 out = 4x mode). 2-phase software pipeline:
phase1(k+1) overlaps phase2(k).

HW-measured mode lessons baked in: every DVE op uses plain 2-operand
views with 4B-aligned even bases where possible — packed multi-segment
APs (stride-0 broadcast, odd-offset segment pairs, multi-PSUM-bank
views) and f8 outputs all fall off the 2x/4x fast paths on real HW, and
gpsimd tensor ops are far slower than the cost model claims.

Layout: partition = image COLUMN, free = ROW; 8 cores x (image, H-half);
7 overlapping 128-column stripes (124 valid cols each).
"""

import sys

sys.path.insert(0, "/opt/trn_rl_repo")

from contextlib import ExitStack

import numpy as np

import concourse.bass as bass
import concourse.tile as tile
from concourse import bacc, mybir
from concourse.bass_utils import run_bass_kernel_spmd

F32 = mybir.dt.float32
F16 = mybir.dt.float16
F8 = mybir.dt.float8e4

B, H, W = 4, 768, 768
NCORES = 8
SLAB = H // 2
NST = 7
CW = 124
XR = SLAB + 4          # 388
SEG = XR + SLAB        # 772
ROWS = SLAB            # 384
BANK = 512

_CACHE = {}


def _consts():
    ident = np.eye(128, dtype=np.float16)
    b1 = np.zeros((128, 128), dtype=np.float16)
    b2 = np.zeros((128, 128), dtype=np.float16)
    for m in range(128):
        for d in (-1, 1):
            if 0 <= m + d < 128:
                b1[m + d, m] = 1.0
        for d in (-2, 2):
            if 0 <= m + d < 128:
                b2[m + d, m] = 1.0
    wpack = np.concatenate([b1, b2, ident], axis=1)
    return {"wpack": wpack}


def _seg2(view, off0, step, n=ROWS):
    """AP over `view`'s tensor: two segments of length n at elem offsets
    off0 and off0+step (outer dim count 2, stride `step`)."""
    base = view[:, off0 : off0 + n]
    return bass.AP(
        base.tensor, base.offset, [list(base.ap[0]), [step, 2], [1, n]]
    )


def _bcast2(view, n=ROWS):
    """[128, n] view broadcast to [128, 2, n] via stride-0 outer dim."""
    return bass.AP(view.tensor, view.offset, [list(view.ap[0]), [0, 2], [1, n]])


def _build(repeat: int = 1):
    nc = bacc.Bacc(
        "TRN2",
        target_bir_lowering=False,
        debug=False,
        enable_asserts=True,
        num_devices=NCORES,
    )
    ins_d = nc.dram_tensor("ins", [128, NST * SEG], F16, kind="ExternalInput").ap()
    wp_d = nc.dram_tensor("wpack", [128, 384], F16, kind="ExternalInput").ap()
    conv_d = nc.dram_tensor("conv", [128, NST * ROWS], F16, kind="ExternalOutput").ap()
    mask_d = nc.dram_tensor("mask", [128, NST * ROWS], F16, kind="ExternalOutput").ap()

    with tile.TileContext(nc, trace_sim=False) as tc, ExitStack() as ctx:
        sb = ctx.enter_context(tc.tile_pool(name="sb", bufs=3))
        cb = ctx.enter_context(tc.tile_pool(name="cb", bufs=1))
        ps = ctx.enter_context(tc.tile_pool(name="ps", bufs=1, space="PSUM"))

        wp = cb.tile([128, 384], F16, tag="wpack")
        nc.sync.dma_start(wp[:], wp_d[:])
        w_b1, w_b2, w_id = wp[:, 0:128], wp[:, 128:256], wp[:, 256:384]

        def _body():
            ins = sb.tile([128, NST * SEG], F16, tag="ins")
            for k in range(0, NST, 2):
                o, o2 = k * SEG, min(NST, k + 2) * SEG
                nc.sync.dma_start(ins[:, o:o2], ins_d[:, o:o2])
            oc = sb.tile([128, NST * ROWS], F16, tag="oc")
            om = sb.tile([128, NST * ROWS], F16, tag="om")

            st = [dict() for _ in range(NST)]

            def phase1(k):
                o = k * SEG
                xk = ins[:, o : o + XR]
                sk = ins[:, o + XR : o + SEG]
                d = st[k]

                xr = sb.tile([128, XR], F16, tag=f"xr{k%2}")
                nc.vector.tensor_scalar_max(xr[:], xk, 0.0)
                d["xr"] = xr
                u0c = xr[:, 2 : 2 + ROWS]

                # u12 = [u1|u2] in one packed DVE add:
                #   u1 = xr[1:385] + xr[3:387]; u2 = xr[0:384] + xr[4:388]
                u12 = sb.tile([128, 2 * ROWS], F16, tag=f"u12{k%2}")
                nc.vector.tensor_add(u12[:, 0:ROWS], xr[:, 1 : 1 + ROWS],
                                     xr[:, 3 : 3 + ROWS])
                nc.vector.tensor_add(u12[:, ROWS : 2 * ROWS], xr[:, 0:ROWS],
                                     xr[:, 4 : 4 + ROWS])
                u1, u2 = u12[:, 0:ROWS], u12[:, ROWS : 2 * ROWS]

                # weights: vr = 1/s^2; t1 = exp(-0.5 vr); t4 = exp(-2 vr)
                q = sb.tile([128, ROWS], F32, tag=f"q{k%2}")
                nc.scalar.activation(q[:], sk, mybir.ActivationFunctionType.Square)
                vr = sb.tile([128, ROWS], F32, tag=f"vr{k%2}")
                nc.vector.reciprocal_approx_fast(vr[:], q[:])
                t14 = sb.tile([128, 2 * ROWS], F16, tag=f"t14{k%2}")
                d["t14"] = t14
                nc.scalar.activation(
                    t14[:, 0:ROWS], vr[:], mybir.ActivationFunctionType.Exp,
                    scale=-0.5,
                )
                nc.scalar.activation(
                    t14[:, ROWS : 2 * ROWS], vr[:],
                    mybir.ActivationFunctionType.Exp, scale=-2.0,
                )

                pair = ps.tile([128, 2 * BANK], F32, tag=f"pair{k%2}")
                trip = ps.tile([128, 3 * BANK], F32, tag="trip")
                p1 = pair[:, 0:ROWS]
                p4 = pair[:, BANK : BANK + ROWS]
                p2 = trip[:, 0:ROWS]
                p5 = trip[:, BANK : BANK + ROWS]
                p8 = trip[:, 2 * BANK : 2 * BANK + ROWS]
                d.update(pair=pair, trip=trip, p1=p1, p4=p4)

                nc.tensor.matmul(p1, w_b1, u0c, start=True, stop=False)
                nc.tensor.matmul(p1, w_id, u1, start=False, stop=False)
                nc.tensor.matmul(p2, w_b1, u1, start=True, stop=True)
                nc.tensor.matmul(p4, w_b2, u0c, start=True, stop=False)
                nc.tensor.matmul(p4, w_id, u2, start=False, stop=False)
                nc.tensor.matmul(p5, w_b2, u1, start=True, stop=False)
                nc.tensor.matmul(p5, w_b1, u2, start=False, stop=True)
                nc.tensor.matmul(p8, w_b2, u2, start=True, stop=True)

            def phase2a(k):
                d = st[k]
                t14 = d["t14"]
                t1 = t14[:, 0:ROWS]
                t4 = t14[:, ROWS : 2 * ROWS]

                # packed ScalarE evac of [P2|P5|P8]
                e = sb.tile([128, 3 * ROWS], F16, tag=f"e{k%2}")
                nc.scalar.copy(
                    e[:].rearrange("p (b r) -> p b r", b=3, r=ROWS),
                    d["trip"][:].rearrange("p (b r) -> p b r", b=3, r=BANK)[
                        :, :, 0:ROWS
                    ],
                )

                # m1 = t1*e2; m2 = t1*e5 (plain 2x muls)
                m12 = sb.tile([128, 2 * ROWS], F16, tag=f"m12{k%2}")
                nc.vector.tensor_mul(m12[:, 0:ROWS], t1, e[:, 0:ROWS])
                nc.vector.tensor_mul(m12[:, ROWS : 2 * ROWS], t1,
                                     e[:, ROWS : 2 * ROWS])
                # m3 = t4 * e8 on gpsimd
                m3 = sb.tile([128, ROWS], F16, tag=f"m3{k%2}")
                nc.vector.tensor_mul(m3[:], t4, e[:, 2 * ROWS : 3 * ROWS])

                # folds: P1 += m1; P4 += m2 + m3
                nc.tensor.matmul(d["p1"], w_id, m12[:, 0:ROWS], start=False, stop=True)
                nc.tensor.matmul(d["p4"], w_id, m12[:, ROWS : 2 * ROWS],
                                 start=False, stop=False)
                nc.tensor.matmul(d["p4"], w_id, m3[:], start=False, stop=True)

            def phase2b(k):
                d = st[k]
                t14 = d["t14"]
                xr = d["xr"]
                u0c = xr[:, 2 : 2 + ROWS]

                # fused evac+mul: mab = [t1|t4] (x) [P1'|P4'] straight from PSUM
                mab = sb.tile([128, 2 * ROWS], F16, tag=f"mab{k%2}")
                nc.vector.tensor_mul(
                    mab[:].rearrange("p (b r) -> p b r", b=2, r=ROWS),
                    t14[:].rearrange("p (b r) -> p b r", b=2, r=ROWS),
                    d["pair"][:].rearrange("p (b r) -> p b r", b=2, r=BANK)[
                        :, :, 0:ROWS
                    ],
                )
                s = sb.tile([128, ROWS], F16, tag=f"s{k%2}")
                nc.vector.tensor_add(s[:], mab[:, 0:ROWS], mab[:, ROWS : 2 * ROWS])

                co = k * ROWS
                nc.vector.tensor_add(oc[:, co : co + ROWS], u0c, s[:])
                nc.vector.tensor_scalar(
                    om[:, co : co + ROWS], oc[:, co : co + ROWS],
                    0.5, None, mybir.AluOpType.is_ge,
                )

            def flush(lo, hi):
                a, b = lo * ROWS, hi * ROWS
                nc.scalar.dma_start(conv_d[:, a:b], oc[:, a:b])
                nc.scalar.dma_start(mask_d[:, a:b], om[:, a:b])

            phase1(0)
            for k in range(NST):
                if k + 1 < NST:
                    phase1(k + 1)
                phase2a(k)
                phase2b(k)

            for k in range(0, NST, 2):
                flush(k, min(k + 2, NST))

        if repeat == 1:
            _body()
        elif repeat % 2 == 0:
            with tc.For_i(0, repeat // 2, 1):
                _body()
                _body()
        else:
            with tc.For_i(0, repeat, 1):
                _body()

    nc.compile()
    return nc


def make_in_maps(bev_map: np.ndarray, bev_scale: np.ndarray):
    consts = _consts()
    in_maps = []
    for c in range(NCORES):
        b, hh = c // 2, c % 2
        xT = np.zeros((124 * NST + 4, 772), dtype=np.float16)
        xT[2:770, 2:770] = bev_map[b, 0].T
        sT = np.ones((124 * NST + 4, 768), dtype=np.float16)
        sT[2:770, :] = bev_scale[b, 0].T
        r0 = hh * SLAB
        ins = np.empty((128, NST * SEG), dtype=np.float16)
        for k in range(NST):
            o = k * SEG
            ins[:, o : o + XR] = xT[124 * k : 124 * k + 128, r0 : r0 + XR]
            ins[:, o + XR : o + SEG] = sT[
                124 * k : 124 * k + 128, r0 : r0 + SLAB
            ]
        m = {"ins": ins}
        m.update({k2: v.copy() for k2, v in consts.items()})
        in_maps.append(m)
    return in_maps


def _unpack(res):
    conv = np.empty((B, 1, H, W), dtype=np.float32)
    mask = np.empty((B, 1, H, W), dtype=np.float32)
    for c in range(NCORES):
        b, hh = c // 2, c % 2
        ocf = np.asarray(res[c]["conv"]).astype(np.float32)
        omf = np.asarray(res[c]["mask"]).astype(np.float32)
        convT = np.empty((W, SLAB), dtype=np.float32)
        maskT = np.empty((W, SLAB), dtype=np.float32)
        for k in range(NST):
            c0 = 124 * k
            n = min(CW, W - c0)
            seg = slice(k * ROWS, (k + 1) * ROWS)
            convT[c0 : c0 + n] = ocf[2 : 2 + n, seg]
            maskT[c0 : c0 + n] = omf[2 : 2 + n, seg]
        r0 = hh * SLAB
        conv[b, 0, r0 : r0 + SLAB, :] = convT.T
        mask[b, 0, r0 : r0 + SLAB, :] = maskT.T
    return conv, mask


def kernel(bev_map: np.ndarray, bev_scale: np.ndarray):
    assert bev_map.shape == (B, 1, H, W) and bev_scale.shape == (B, 1, H, W)
    if "nc" not in _CACHE:
        _CACHE["nc"] = _build()
    nc = _CACHE["nc"]
    in_maps = make_in_maps(bev_map, bev_scale)
    res = run_bass_kernel_spmd(nc, in_maps, list(range(NCORES))).results
    return _unpack(res)
